# revision 1
# baseline (speedup 1.0000x reference)
"""Trainium2 Bass kernel for nn_NbrAttn2 (neighbor cross-attention block).

Sharding: 8 cores = 4 batches x 2 kv-halves. Each core computes the full
attention for its batch over half the neighbors (kv = 8*512 = 4096),
producing unnormalized per-head context + softmax denominators; core pairs
AllReduce-add these, then each core normalizes, output-projects, adds the
residual and writes the full [T, D] output for its batch.
"""

import math

import numpy as np

B, T, N, D, H = 4, 512, 16, 256, 8
DK = D // H  # 32
CTS, CN, CE = 6, 4, 3
TSE, AUXE = 192, 64
NCORES = 8
NBH = N // 2       # neighbors per core
KV = NBH * T       # 4096 kv positions per core
KC = KV // 128     # 32 kv chunks of 128
F32MAX = None

_CACHE = {}


def _pe_table() -> np.ndarray:
    # matches reference.pe_table numerics (fp32)
    pos = np.arange(T, dtype=np.float32)[:, None]
    div = np.exp(
        np.arange(0, D, 2, dtype=np.float32)
        * (np.float32(-np.log(np.float32(10000.0))) / np.float32(D))
    ).astype(np.float32)
    pe = np.zeros((T, D), dtype=np.float32)
    pe[:, 0::2] = np.sin(pos * div)
    pe[:, 1::2] = np.cos(pos * div)
    return pe


def _hmap(h):
    """head -> (tile index, partition offset) over 96/96/64 partition tiles."""
    return (h // 3, (h % 3) * DK) if h < 6 else (2, (h - 6) * DK)


def build_nc(loop: int = 0, no_collective: bool = False, phases: str = 'mlpa'):
    import contextlib

    import concourse.bass as bass
    import concourse.mybir as mybir
    import concourse.tile as tile
    from concourse import bacc
    from concourse.masks import make_identity

    f32 = mybir.dt.float32
    bf16 = mybir.dt.bfloat16
    i32 = mybir.dt.int32
    AF = mybir.ActivationFunctionType
    OP = mybir.AluOpType

    nc = bacc.Bacc()

    # ---- DRAM parameters (per-core shapes) ----
    dp = nc.declare_dram_parameter
    maskT_h = dp("maskt", [KV, T], i32, isOutput=False)
    x_h = dp("xb", [T, D], f32, isOutput=False)
    pe_h = dp("pe", [T, D], f32, isOutput=False)
    md_h = dp("md", [NBH, CTS, T], f32, isOutput=False)
    na_h = dp("na", [NBH, CN, T], f32, isOutput=False)
    ea_h = dp("ea", [NBH, CE, T], f32, isOutput=False)
    wts1_h = dp("wts1", [CTS, TSE], f32, isOutput=False)
    wts2_h = dp("wts2", [TSE, TSE], f32, isOutput=False)
    wa1_h = dp("wa1", [CN, AUXE], f32, isOutput=False)
    wa2_h = dp("wa2", [AUXE, AUXE], f32, isOutput=False)
    we1_h = dp("we1", [CE, D], f32, isOutput=False)
    we2_h = dp("we2", [D, D], f32, isOutput=False)
    wq_h = dp("wq", [D, D], f32, isOutput=False)       # /sqrt(DK), bias separate
    bias_h = dp("biases", [6, D, 1], f32, isOutput=False)  # ts1,ts2,a...,e2,bqs packed
    wk_h = dp("wk", [D, D], f32, isOutput=False)       # bias folded into pewk
    wv_h = dp("wv", [D, D], f32, isOutput=False)       # bias folded into pewv
    wo_h = dp("wo", [D + 1, D], f32, isOutput=False)   # bias row incl.
    pewk_h = dp("pewk", [D, T], f32, isOutput=False)   # (pe@Wk + bk).T
    pewv_h = dp("pewv", [T, D], f32, isOutput=False)   # pe@Wv + bv
    lng_h = dp("lng", [D, 1], f32, isOutput=False)
    lnb_h = dp("lnb", [D, 1], f32, isOutput=False)
    out_h = dp("out", [T, D], f32, isOutput=True)

    RG = [[0, 1], [2, 3], [4, 5], [6, 7]]

    with tile.TileContext(nc, num_cores=NCORES) as tc:
        with (
            tc.tile_pool(name="const", bufs=1) as const,
            tc.tile_pool(name="big", bufs=1) as big,
            tc.tile_pool(name="stage", bufs=1) as stage,
            tc.tile_pool(name="prep", bufs=2) as prep,
            tc.tile_pool(name="ppool", bufs=3) as ppool,
            tc.tile_pool(name="pwork", bufs=2, space="PSUM") as pwork,
            tc.tile_pool(name="psco", bufs=2, space="PSUM") as psco,
            tc.tile_pool(name="pctx", bufs=2, space="PSUM") as pctx,
            tc.tile_pool(name="dram", bufs=1, space="DRAM") as dram,
        ):
            # ---------------- constants ----------------
            ident = const.tile([128, 128], f32, name="ident")
            make_identity(nc, ident[:])
            ones_row = const.tile([1, T], f32, name="ones_row")
            nc.vector.memset(ones_row[:], 1.0)
            eps_col = const.tile([128, 1], f32, name="eps_col")
            nc.vector.memset(eps_col[:], 1e-6)

            def load_const(name, src, p, f):
                t = const.tile([p, f], f32, name=name)
                nc.sync.dma_start(out=t[:], in_=src)
                return t

            w_ts1 = load_const("w_ts1", wts1_h[:], CTS, TSE)
            w_ts2_c0 = load_const("w_ts2_c0", wts2_h[0:128], 128, TSE)
            w_ts2_c1 = load_const("w_ts2_c1", wts2_h[128:TSE], TSE - 128, TSE)
            w_a1 = load_const("w_a1", wa1_h[:], CN, AUXE)
            w_a2 = load_const("w_a2", wa2_h[:], AUXE, AUXE)
            w_e1 = load_const("w_e1", we1_h[:], CE, D)
            w_e2_c0 = load_const("w_e2_c0", we2_h[0:128], 128, D)
            w_e2_c1 = load_const("w_e2_c1", we2_h[128:256], 128, D)
            w_q_c0 = load_const("w_q_c0", wq_h[0:128], 128, D)
            w_q_c1 = load_const("w_q_c1", wq_h[128:256], 128, D)
            # bias columns: 0=b_ts1, 1=b_ts2, 2=b_a (a1 in 0:64, a2 in 64:128),
            # 3=b_e1, 4=b_e2, 5=bq/sqrt(DK)
            _bc = [(load_const(f"bcol{i}a", bias_h[i, 0:128], 128, 1),
                    load_const(f"bcol{i}b", bias_h[i, 128:256], 128, 1))
                   for i in range(6)]

            class _BCol:
                def __init__(self, pair):
                    self.pair = pair

                def __getitem__(self, s):
                    lo, hi = s.start or 0, s.stop
                    c, r = divmod(lo, 128)
                    assert hi - lo <= 128 - r
                    return self.pair[c][r : r + (hi - lo)]

            bcol = [_BCol(p) for p in _bc]
            w_k_c0 = load_const("w_k_c0", wk_h[0:128], 128, D)
            w_k_c1 = load_const("w_k_c1", wk_h[128:256], 128, D)
            w_v_c0 = load_const("w_v_c0", wv_h[0:128], 128, D)
            w_v_c1 = load_const("w_v_c1", wv_h[128:256], 128, D)
            w_o_c0 = load_const("w_o_c0", wo_h[0:128], 128, D)
            w_o_c1 = load_const("w_o_c1", wo_h[128:256], 128, D)
            w_o_b = load_const("w_o_b", wo_h[256:257], 1, D)
            pewk = [load_const(f"pewk{c}", pewk_h[c * 128 : (c + 1) * 128], 128, T)
                    for c in range(2)]
            pewv = [load_const(f"pewv{j}", pewv_h[j * 128 : (j + 1) * 128], 128, D)
                    for j in range(4)]
            lng = [load_const(f"lng{c}", lng_h[c * 128 : (c + 1) * 128], 128, 1)
                   for c in range(2)]
            lnb = [load_const(f"lnb{c}", lnb_h[c * 128 : (c + 1) * 128], 128, 1)
                   for c in range(2)]

            # ---------------- persistent big tensors ----------------
            mask_bf = big.tile([128, KC * T], bf16, name="mask_bf")
            kT_bf = [big.tile([96, KV], bf16, name="kT_a"),
                     big.tile([96, KV], bf16, name="kT_b"),
                     big.tile([64, KV], bf16, name="kT_c")]
            qT_bf = [big.tile([96, T], bf16, name="qT_a"),
                     big.tile([96, T], bf16, name="qT_b"),
                     big.tile([64, T], bf16, name="qT_c")]
            v_aug = big.tile([128, KC * (D + H)], bf16, name="v_aug")
            xq_sb = [big.tile([128, D], f32, name=f"xq{t}") for t in range(4)]
            xnT = [big.tile([128, T], f32, name=f"xnT{c}") for c in range(2)]
            ctxT_sb = [big.tile([128, T], f32, name=f"ctxT{c}") for c in range(2)]
            z_sb = big.tile([1, H * T], f32, name="z_sb")
            ctx_acc = [big.tile([128, T], f32, name=f"ctxacc{i}") for i in range(4)]
            red_c = xnT          # reuse: xnT is dead after q projection
            red_z = z_sb         # reuse: z_sb is dead after cc_in DMA
            ctxn = ctxT_sb       # reuse: partials dead after collective

            # v_aug layout: [128, kc, H, DK+1]; set the ones column once
            va4 = v_aug.rearrange("p (c h e) -> p c h e", c=KC, h=H)
            nc.vector.memset(va4[:, :, :, DK : DK + 1], 1.0)
            va3 = v_aug.rearrange("p (c e) -> p c e", c=KC)  # [128, KC, 264]

            if "a" in phases and "p" not in phases:  # timing-only variants
                for t_ in kT_bf + qT_bf:
                    nc.vector.memset(t_[:], 0.5)
                nc.vector.memset(v_aug[:], 0.5)
            if "a" in phases and "m" not in phases:
                nc.vector.memset(mask_bf[:], 1.0)
            if "l" not in phases:
                for t_ in xq_sb + xnT:
                    nc.vector.memset(t_[:], 0.0)

            for _rep in range(max(1, loop)):
                for i in range(4 if 'a' in phases else 0):
                    nc.vector.memset(ctx_acc[i][:], 0.0)
                # ---------------- mask load + convert ----------------
                mT = maskT_h.rearrange("(c p) t -> p c t", p=128)  # [128, KC, T]
                for mc in range(16 if 'm' in phases else 0):
                    st = stage.tile([128, 2 * T], i32, name="mstage")
                    nc.sync.dma_start(out=st[:], in_=mT[:, mc * 2 : (mc + 1) * 2, :])
                    nc.gpsimd.tensor_copy(
                        mask_bf[:, mc * 2 * T : (mc + 1) * 2 * T], st[:]
                    )

                # ---------------- layernorm + q ----------------
                for t in range(4 if 'l' in phases else 0):
                    xt = prep.tile([128, D], f32, name="lnw", tag="lnw", bufs=4)
                    nc.sync.dma_start(out=xt[:], in_=x_h[t * 128 : (t + 1) * 128])
                    pet = prep.tile([128, D], f32, name="pet", tag="lnw", bufs=4)
                    nc.sync.dma_start(out=pet[:], in_=pe_h[t * 128 : (t + 1) * 128])
                    nc.vector.tensor_add(xq_sb[t][:], xt[:], pet[:])
                    mu = prep.tile([128, 1], f32, name="ln_mu", tag="lncol", bufs=8)
                    nc.vector.tensor_reduce(
                        mu[:], xq_sb[t][:], mybir.AxisListType.X, OP.add
                    )
                    nc.vector.tensor_scalar_mul(mu[:], mu[:], 1.0 / D)
                    xc = prep.tile([128, D], f32, name="ln_xc", tag="lnw", bufs=4)
                    nc.vector.tensor_scalar(xc[:], xq_sb[t][:], mu[:], None, OP.subtract)
                    sq = prep.tile([128, D], f32, name="ln_sq", tag="lnw", bufs=4)
                    var = prep.tile([128, 1], f32, name="ln_var", tag="lncol", bufs=8)
                    nc.scalar.activation(sq[:], xc[:], AF.Square, accum_out=var[:])
                    std = prep.tile([128, 1], f32, name="ln_std", tag="lncol", bufs=8)
                    nc.scalar.activation(std[:], var[:], AF.Sqrt, bias=eps_col[:], scale=1.0 / D)
                    rstd = prep.tile([128, 1], f32, name="ln_rstd", tag="lncol", bufs=8)
                    nc.vector.reciprocal(rstd[:], std[:])
                    xn0 = prep.tile([128, D], f32, name="ln_xn0", tag="lnw", bufs=4)
                    nc.vector.tensor_scalar_mul(xn0[:], xc[:], rstd[:])
                    for c in range(2):
                        tp = pwork.tile([128, 128], f32, name="tp", tag="pw")
                        nc.tensor.transpose(
                            tp[:], xn0[:, c * 128 : (c + 1) * 128], ident[:]
                        )
                        nc.vector.tensor_scalar(
                            xnT[c][:, t * 128 : (t + 1) * 128],
                            tp[:], lng[c][:], lnb[c][:], OP.mult, OP.add,
                        )
                for mchunk in range(2 if 'l' in phases else 0):
                    qp = pwork.tile([128, T], f32, name="qpsum", tag="pw")
                    ms = slice(mchunk * 128, (mchunk + 1) * 128)
                    nc.tensor.matmul(qp[:], w_q_c0[:, ms], xnT[0][:], start=True, stop=False)
                    nc.tensor.matmul(qp[:], w_q_c1[:, ms], xnT[1][:], start=False, stop=True)
                    bq_ = bcol[5]
                    if mchunk == 0:
                        nc.vector.tensor_scalar_add(qT_bf[0][:], qp[0:96, :], bq_[0:96])
                        nc.vector.tensor_scalar_add(qT_bf[1][0:32, :], qp[96:128, :], bq_[96:128])
                    else:
                        nc.vector.tensor_scalar_add(qT_bf[1][32:64, :], qp[0:32, :], bq_[128:160])
                        nc.vector.tensor_scalar_add(qT_bf[1][64:96, :], qp[32:64, :], bq_[160:192])
                        nc.vector.tensor_scalar_add(qT_bf[2][:], qp[64:128, :], bq_[192:256])

                # ---------------- per-neighbor kv prep ----------------
                for n in range(NBH if ('p' in phases or 'a' in phases) else 0):
                    do_prep = 'p' in phases

                    if do_prep:
                        md_rhs = prep.tile([CTS, T], f32, name="md_rhs", tag="rhs_in", bufs=4)
                        nc.sync.dma_start(out=md_rhs[:], in_=md_h[n])
                        na_rhs = prep.tile([CN, T], f32, name="na_rhs", tag="rhs_in", bufs=4)
                        nc.sync.dma_start(out=na_rhs[:], in_=na_h[n])
                        ea_rhs = prep.tile([CE, T], f32, name="ea_rhs", tag="rhs_in", bufs=4)
                        nc.sync.dma_start(out=ea_rhs[:], in_=ea_h[n])

                        # ts branch
                        ts1_c0 = prep.tile([128, T], f32, name="ts1_c0", tag="ts1", bufs=2)
                        ts1_c1 = prep.tile([TSE - 128, T], f32, name="ts1_c1", tag="ts1", bufs=2)
                        ps = pwork.tile([128, T], f32, name="prep_ps", tag="pw")
                        nc.tensor.matmul(ps[:], w_ts1[:, 0:128], md_rhs[:], start=True, stop=True)
                        nc.scalar.activation(ts1_c0[:], ps[:], AF.Relu, bias=bcol[0][0:128])
                        ps = pwork.tile([128, T], f32, name="prep_ps", tag="pw")
                        nc.tensor.matmul(
                            ps[0 : TSE - 128, :], w_ts1[:, 128:TSE], md_rhs[:],
                            start=True, stop=True,
                        )
                        nc.scalar.activation(
                            ts1_c1[:], ps[0 : TSE - 128, :], AF.Relu, bias=bcol[0][128:TSE]
                        )

                        # nbrT = [ts2 (192) ; a2 (64)] as 2 chunks of 128
                        nbr_c0 = prep.tile([128, T], f32, name="nbr_c0", tag="nbr", bufs=3)
                        nbr_c1 = prep.tile([128, T], f32, name="nbr_c1", tag="nbr", bufs=3)
                        ps = pwork.tile([128, T], f32, name="prep_ps", tag="pw")
                        nc.tensor.matmul(ps[:], w_ts2_c0[:, 0:128], ts1_c0[:], start=True, stop=False)
                        nc.tensor.matmul(ps[:], w_ts2_c1[:, 0:128], ts1_c1[:], start=False, stop=True)
                        nc.vector.tensor_scalar_add(nbr_c0[:], ps[:], bcol[1][0:128])
                        ps = pwork.tile([128, T], f32, name="prep_ps", tag="pw")
                        nc.tensor.matmul(
                            ps[0:64, :], w_ts2_c0[:, 128:TSE], ts1_c0[:], start=True, stop=False
                        )
                        nc.tensor.matmul(
                            ps[0:64, :], w_ts2_c1[:, 128:TSE], ts1_c1[:], start=False, stop=True
                        )
                        nc.vector.tensor_scalar_add(nbr_c1[0:64, :], ps[0:64, :], bcol[1][128:TSE])

                        # aux branch
                        a1 = prep.tile([AUXE, T], f32, name="a1sb", tag="ts1", bufs=2)
                        ps = pwork.tile([128, T], f32, name="prep_ps", tag="pw")
                        nc.tensor.matmul(ps[0:AUXE, :], w_a1[:], na_rhs[:], start=True, stop=True)
                        nc.scalar.activation(a1[:], ps[0:AUXE, :], AF.Relu, bias=bcol[2][0:AUXE])
                        ps = pwork.tile([128, T], f32, name="prep_ps", tag="pw")
                        nc.tensor.matmul(ps[0:AUXE, :], w_a2[:], a1[:], start=True, stop=True)
                        nc.vector.tensor_scalar_add(
                            nbr_c1[64:128, :], ps[0:AUXE, :], bcol[2][64:128]
                        )

                        # edge branch
                        e1 = [prep.tile([128, T], f32, name=f"e1_{c}", tag="e1", bufs=2)
                              for c in range(2)]
                        for c in range(2):
                            ps = pwork.tile([128, T], f32, name="prep_ps", tag="pw")
                            nc.tensor.matmul(
                                ps[:], w_e1[:, c * 128 : (c + 1) * 128], ea_rhs[:],
                                start=True, stop=True,
                            )
                            nc.scalar.activation(
                                e1[c][:], ps[:], AF.Relu, bias=bcol[3][c * 128 : (c + 1) * 128]
                            )
                        e2 = [prep.tile([128, T], f32, name=f"e2_{c}", tag="e2", bufs=2)
                              for c in range(2)]
                        for c in range(2):
                            ms = slice(c * 128, (c + 1) * 128)
                            ps = pwork.tile([128, T], f32, name="prep_ps", tag="pw")
                            nc.tensor.matmul(ps[:], w_e2_c0[:, ms], e1[0][:], start=True, stop=False)
                            nc.tensor.matmul(ps[:], w_e2_c1[:, ms], e1[1][:], start=False, stop=True)
                            nc.vector.tensor_scalar_add(e2[c][:], ps[:], bcol[4][ms])

                        # keysT = nbrT * e2T (pe+bias handled via pewk)
                        keys = [prep.tile([128, T], f32, name=f"keys_{c}", tag="keys", bufs=2)
                                for c in range(2)]
                        nc.gpsimd.tensor_tensor(keys[0][:], nbr_c0[:], e2[0][:], OP.mult)
                        nc.gpsimd.tensor_tensor(keys[1][:], nbr_c1[:], e2[1][:], OP.mult)

                        # kT (bf16): Wk^T keysT + (pe@Wk + bk)^T, split 96/32 & 64/64
                        nts = slice(n * T, (n + 1) * T)
                        for c in range(2):
                            ms = slice(c * 128, (c + 1) * 128)
                            ps = pwork.tile([128, T], f32, name="prep_ps", tag="pw")
                            nc.tensor.matmul(ps[:], w_k_c0[:, ms], keys[0][:], start=True, stop=False)
                            nc.tensor.matmul(ps[:], w_k_c1[:, ms], keys[1][:], start=False, stop=True)
                            if c == 0:
                                nc.vector.tensor_tensor(
                                    kT_bf[0][:, nts], ps[0:96, :], pewk[0][0:96, :], OP.add)
                                nc.vector.tensor_tensor(
                                    kT_bf[1][0:32, nts], ps[96:128, :], pewk[0][96:128, :], OP.add)
                            else:
                                nc.vector.tensor_tensor(
                                    kT_bf[1][32:64, nts], ps[0:32, :], pewk[1][0:32, :], OP.add)
                                nc.vector.tensor_tensor(
                                    kT_bf[1][64:96, nts], ps[32:64, :], pewk[1][32:64, :], OP.add)
                                nc.vector.tensor_tensor(
                                    kT_bf[2][:, nts], ps[64:128, :], pewk[1][64:128, :], OP.add)

                        # v rows (natural layout) = nbr^T Wv + (pe@Wv + bv) -> v_aug
                        for j in range(4):
                            ps = pwork.tile([128, D], f32, name="prep_ps", tag="pw")
                            ts_ = slice(j * 128, (j + 1) * 128)
                            nc.tensor.matmul(ps[:, 0:D], nbr_c0[:, ts_], w_v_c0[:], start=True, stop=False)
                            nc.tensor.matmul(ps[:, 0:D], nbr_c1[:, ts_], w_v_c1[:], start=False, stop=True)
                            kc = n * 4 + j
                            nc.vector.tensor_tensor(
                                va4[:, kc, :, 0:DK],
                                ps[:, 0:D].rearrange("p (h e) -> p h e", h=H),
                                pewv[j][:].rearrange("p (h e) -> p h e", h=H),
                                OP.add,
                            )

                    # ---- fused attention burst over this neighbor's 4 kv chunks ----
                    for hg in range(2 if 'a' in phases else 0):
                        pms = {}
                        for h in range(4 * hg, 4 * hg + 4):
                            hti, hoff = _hmap(h)
                            krow = slice(hoff, hoff + DK)
                            for cc in range(2):
                                sp = psco.tile([128, 2 * T], f32, name="s_ps", tag="sps")
                                for j in range(2):
                                    kc = 4 * n + 2 * cc + j
                                    nc.tensor.matmul(
                                        sp[:, j * T : (j + 1) * T],
                                        kT_bf[hti][krow, kc * 128 : (kc + 1) * 128],
                                        qT_bf[hti][krow, :],
                                        start=True, stop=True,
                                    )
                                p0 = ppool.tile([128, 2 * T], bf16, name="p0", bufs=3)
                                nc.scalar.activation(p0[:], sp[:], AF.Exp)
                                pm = ppool.tile([128, 2 * T], bf16, name="pm", bufs=9)
                                nc.vector.tensor_tensor(
                                    pm[:], p0[:],
                                    mask_bf[:, (4 * n + 2 * cc) * T : (4 * n + 2 * cc + 2) * T],
                                    OP.mult,
                                )
                                pms[2 * h + cc] = pm
                        for h in range(4 * hg, 4 * hg + 4):
                            cp = pctx.tile([DK + 1, T], f32, name="ctx_ps", tag="ctx")
                            for cc in range(2):
                                pm = pms[2 * h + cc]
                                for j in range(2):
                                    kc = 4 * n + 2 * cc + j
                                    nc.tensor.matmul(
                                        cp[:],
                                        va3[:, kc, h * (DK + 1) : (h + 1) * (DK + 1)],
                                        pm[:, j * T : (j + 1) * T],
                                        start=(cc == 0 and j == 0),
                                        stop=(cc == 1 and j == 1),
                                    )
                            off = 64 * (h % 2)
                            acc = ctx_acc[h // 2]
                            nc.vector.tensor_tensor(
                                acc[off : off + DK + 1, :], acc[off : off + DK + 1, :],
                                cp[:], OP.add,
                            )

            # ---------------- split ctx accumulators into ctxT/z ----------------
            for h in range(H if 'a' in phases else 0):
                off = 64 * (h % 2)
                acc = ctx_acc[h // 2]
                c4, r4 = divmod(h, 4)
                nc.vector.tensor_copy(
                    ctxT_sb[c4][r4 * DK : (r4 + 1) * DK, :], acc[off : off + DK, :]
                )
                nc.vector.tensor_copy(
                    z_sb[0:1, h * T : (h + 1) * T], acc[off + DK : off + DK + 1, :]
                )

            # ---------------- pair AllReduce + epilogue ----------------
            do_epi = 'z' in phases or phases == 'mlpa'
            if do_epi:
                cc_in = dram.tile([(2 * 128 + H) * T], f32, name="cc_in")
                cc_out = dram.tile([(2 * 128 + H) * T], f32, name="cc_out")
                cci = cc_in.rearrange("(p t) -> p t", t=T)
                cco = cc_out.rearrange("(p t) -> p t", t=T)
                nc.sync.dma_start(out=cci[0:128], in_=ctxT_sb[0][:])
                nc.sync.dma_start(out=cci[128:256], in_=ctxT_sb[1][:])
                nc.sync.dma_start(
                    out=cc_in[256 * T : (256 + H) * T].rearrange("(o t) -> o t", o=1),
                    in_=z_sb[:],
                )
                if no_collective:
                    nc.sync.dma_start(out=cc_out[:], in_=cc_in[:])
                else:
                    nc.gpsimd.collective_compute(
                        "AllReduce", OP.add, replica_groups=RG,
                        ins=[cc_in[:]], outs=[cc_out[:]],
                    )
                nc.sync.dma_start(out=red_c[0][:], in_=cco[0:128])
                nc.sync.dma_start(out=red_c[1][:], in_=cco[128:256])
                nc.sync.dma_start(
                    out=red_z[:],
                    in_=cc_out[256 * T : (256 + H) * T].rearrange("(o t) -> o t", o=1),
                )
                for h in range(H):
                    hc, hr = divmod(h, 4)
                    rz_t = prep.tile([1, T], f32, name="rz_t", tag="rz", bufs=2)
                    nc.vector.reciprocal(rz_t[:], red_z[0:1, h * T : (h + 1) * T])
                    bc = pwork.tile([DK, T], f32, name="bc_ps", tag="pw")
                    nc.tensor.matmul(
                        bc[:], ones_row[0:1, 0:DK], rz_t[:], start=True, stop=True
                    )
                    nc.vector.tensor_tensor(
                        ctxn[hc][hr * DK : (hr + 1) * DK, :],
                        red_c[hc][hr * DK : (hr + 1) * DK, :],
                        bc[:], OP.mult,
                    )
                for t in range(4):
                    ts_ = slice(t * 128, (t + 1) * 128)
                    op_ = pwork.tile([128, D], f32, name="out_ps", tag="pw")
                    nc.tensor.matmul(op_[:], ctxn[0][:, ts_], w_o_c0[:], start=True, stop=False)
                    nc.tensor.matmul(op_[:], ctxn[1][:, ts_], w_o_c1[:], start=False, stop=False)
                    nc.tensor.matmul(op_[:], ones_row[0:1, ts_], w_o_b[:], start=False, stop=True)
                    ot = prep.tile([128, D], f32, name="out_sb", tag="lnw", bufs=4)
                    nc.vector.tensor_add(ot[:], op_[:], xq_sb[t][:])
                    nc.sync.dma_start(out=out_h[ts_, :], in_=ot[:])


    nc.finalize()
    return nc


def _host_inputs(inputs):
    """Build the 8 per-core input maps from full inputs."""
    pe = _pe_table()
    sc = np.float32(1.0 / math.sqrt(DK))

    def aug(w, b):
        return np.concatenate([w, b[None, :]], axis=0).astype(np.float32)

    w = {k: np.asarray(v, dtype=np.float32) if np.asarray(v).dtype != np.int32
         else np.asarray(v) for k, v in inputs.items()}

    def pad_col(v):
        out = np.zeros((D, 1), np.float32)
        out[: v.shape[0], 0] = v
        return out

    biases = np.stack([
        pad_col(w["b_ts1"]),
        pad_col(w["b_ts2"]),
        pad_col(np.concatenate([w["b_a1"], w["b_a2"]])),
        pad_col(w["b_e1"]),
        pad_col(w["b_e2"]),
        pad_col(w["bq"] * sc),
    ])

    shared = {
        "pe": pe,
        "wts1": w["W_ts1"].astype(np.float32),
        "wts2": w["W_ts2"].astype(np.float32),
        "wa1": w["W_a1"].astype(np.float32),
        "wa2": w["W_a2"].astype(np.float32),
        "we1": w["W_e1"].astype(np.float32),
        "we2": w["W_e2"].astype(np.float32),
        "wq": (w["Wq"] * sc).astype(np.float32),
        "biases": biases,
        "wk": w["Wk"],
        "wv": w["Wv"],
        "wo": aug(w["Wo"], w["bo"]),
        "pewk": np.ascontiguousarray((pe @ w["Wk"] + w["bk"]).T.astype(np.float32)),
        "pewv": (pe @ w["Wv"] + w["bv"]).astype(np.float32),
        "lng": w["ln_g"].reshape(D, 1).astype(np.float32),
        "lnb": w["ln_b"].reshape(D, 1).astype(np.float32),
    }
    in_maps = []
    for c in range(NCORES):
        b, half = divmod(c, 2)
        n0 = half * NBH
        m = dict(shared)
        m["xb"] = w["x"][b]
        m["md"] = w["masked_data"][b, n0 : n0 + NBH]
        m["na"] = w["node_aux"][b, n0 : n0 + NBH]
        m["ea"] = w["edge_aux"][b, n0 : n0 + NBH]
        m["maskt"] = np.ascontiguousarray(
            w["attention_mask"][b, :, half * KV : (half + 1) * KV].T
        )
        in_maps.append(m)
    return in_maps


def _get_nc():
    if "nc" not in _CACHE:
        _CACHE["nc"] = build_nc()
    return _CACHE["nc"]


def kernel(**inputs) -> np.ndarray:
    from concourse.bass_utils import run_bass_kernel_spmd

    nc = _get_nc()
    in_maps = _host_inputs(inputs)
    res = run_bass_kernel_spmd(nc, in_maps, list(range(NCORES)))
    out = np.stack([res.results[2 * b]["out"] for b in range(B)], axis=0)
    return out.astype(np.float32)

